# revision 26
# baseline (speedup 1.0000x reference)
"""BPCA Unpooling kernel for Trainium2 (8 NeuronCores, data-parallel over batch).

Math per sample s (reference semantics):
    _, s_, vh = svd(X)            # X: [N=65536, 16]
    orig = X @ vh
    out  = orig * std(orig, axis=0) + mean(orig, axis=0)   -> reshape [64,64,256]

Identities (same as the f32 baseline): out = X @ W + mean with W = vh * std,
mean/std computed in closed form from the SVD factors on host.  The SVD runs
on host via jax-CPU (LAPACK sgesdd sign conventions must match the reference).

Device formulation ("Y^T layout"): host pre-transposes X to XT [16, N],
converts to bf16, and packs it into per-core DRAM tiles so that each
[128, 512] sub-tile ("group", 4096 rows) R[(m,k), f] = XT[k, n0 + 512m + f].
A single matmul per group with stationary lhsT = kron(I8, W) gives
    P[(m,j), f] = sum_k W[k,j] X[n0+512m+f, k] = Y[n0+512m+f, j]
so output tiles DMA back to DRAM contiguously in the same packed layout,
which host unpacks to Y [N, 16] f32.

This removes the PE transpose pass and the PSUM->SBUF copy of the f32
baseline, and bf16 in/out halves HBM traffic (the binding constraint):
per core 8 MiB in + 8 MiB out ~= 43 us at the ~390 GB/s/core effective DMA
rate, plus ~9 us fixed NEFF startup.

DMA plan (measured):
  - each DIRECT2D dma_start costs ~0.9 us on the issuing sequencer and each
    DMA completion adds ~0.3 us to ring 15 (last ring of the stripe), so the
    steady-state stream uses big 1 MiB (8-group) DMAs;
  - head and tail use small 2-group (256 KiB) DMAs so the first matmul isn't
    gated on a fat chunk and the drain after the last add is fine-grained;
  - only plain 2D [128, F] tiles (contiguous in DRAM) stripe evenly across
    all 16 rings (3D APs were measured to use half the rings), hence one
    DRAM parameter per chunk-size class;
  - w/bias const DMAs are issued by sync BEFORE the input stream so PE's
    weights arrive with the first input tile;
  - SBUF holds a dedicated [128, 512] slot per group for both streams
    (128 KiB/partition total), so there are no slot-reuse waits and the
    semaphore count stays small.

The bias add + f32->bf16 downcast (PSUM -> SBUF) alternates between the DVE
(tensor_scalar add) and the scalar/ACT engine (activation Identity with a
per-partition bias AP).  The gpsimd engine issues output DMAs.

Raw Bass (explicit per-engine programs + semaphores), as walrus only allows
one attached sync-wait per Matmult.
"""

import sys

import numpy as np

sys.path.insert(0, "/opt/trn_rl_repo")

B = 32
N = 65536
NC = 16
CORES = 8
SPC = B // CORES          # samples per core
GPS = 16                  # groups per sample
G = SPC * GPS             # 64 groups per core
FREE = 512
M = 8                     # 512-row blocks per group

# chunk schedules, in groups (each group = 128 KiB bf16)
IN_CHUNKS = [4, 4, 8, 8, 8, 8, 8, 8, 4, 2, 1, 1]
OUT_CHUNKS = [8, 8, 8, 8, 8, 8, 8, 4, 2, 1, 1]
# Queue assignment: sync issues the input stream, gpsimd the output stream
# (balanced 8 MiB each).  The final out-chunk goes on sync (idle by then)
# so the last two DIRECT2D issues overlap instead of serializing on gpsimd.
GP_IN = []
GP_OUT = list(range(len(OUT_CHUNKS) - 1))
assert sum(IN_CHUNKS) == G and sum(OUT_CHUNKS) == G

IBG = 64   # in-tile group slots: one per group, so no slot-reuse waits
OTG = 64   # out-tile group slots: one per group
OB = 6     # matmul PSUM banks
LIN = 8    # in-DMA completion semaphores (chunks >= 8 apart never overlap)


def _starts(chunks):
    s, out = 0, []
    for c in chunks:
        out.append(s)
        s += c
    return out


IN_STARTS = _starts(IN_CHUNKS)
OUT_STARTS = _starts(OUT_CHUNKS)
for _s, _c in zip(IN_STARTS, IN_CHUNKS):
    assert _s % IBG + _c <= IBG and _s % GPS + _c <= GPS
for _s, _c in zip(OUT_STARTS, OUT_CHUNKS):
    assert _s % OTG + _c <= OTG and _s % GPS + _c <= GPS


def _classes(chunks):
    """chunk list -> {size: count}, and per-chunk (size, index-within-size)."""
    counts, refs = {}, []
    for c in chunks:
        i = counts.get(c, 0)
        refs.append((c, i))
        counts[c] = i + 1
    return counts, refs


IN_COUNTS, IN_REFS = _classes(IN_CHUNKS)
OUT_COUNTS, OUT_REFS = _classes(OUT_CHUNKS)

TRACE = False             # test.py sets this for profiling runs
LAST_EXEC_NS = None       # filled when TRACE

_compiled = None


def _build_graph():
    import concourse.bass as bass
    import concourse.mybir as mybir

    f32 = mybir.dt.float32
    bf16 = mybir.dt.bfloat16

    nc = bass.Bass()

    w_d = nc.declare_dram_parameter("w", [128, SPC * 128], bf16, isOutput=False)
    b_d = nc.declare_dram_parameter("bias", [128, SPC], f32, isOutput=False)
    x_cls = {
        c: nc.declare_dram_parameter(f"x{c}", [n, 128, c * FREE], bf16, isOutput=False)
        for c, n in IN_COUNTS.items()
    }
    o_cls = {
        c: nc.declare_dram_parameter(f"o{c}", [n, 128, c * FREE], bf16, isOutput=True)
        for c, n in OUT_COUNTS.items()
    }

    from contextlib import ExitStack

    with ExitStack() as ctx:
        w_sb = ctx.enter_context(nc.sbuf_tensor([128, SPC * 128], bf16))
        bias_sb = ctx.enter_context(nc.sbuf_tensor([128, SPC], f32))
        in_t = ctx.enter_context(nc.sbuf_tensor([128, IBG * FREE], bf16))
        ot_t = ctx.enter_context(nc.sbuf_tensor([128, OTG * FREE], bf16))
        op = [ctx.enter_context(nc.psum_tensor(f"op{i}", [128, FREE], f32)) for i in range(OB)]
        s_const = ctx.enter_context(nc.semaphore())
        s_mm = ctx.enter_context(nc.semaphore())
        s_add_e = ctx.enter_context(nc.semaphore())
        s_add_o = ctx.enter_context(nc.semaphore())
        s_in = [ctx.enter_context(nc.semaphore(f"s_in{i}")) for i in range(LIN)]
        # out-DMA completions have no waiter (every group has its own slot);
        # a single semaphore absorbs the increments
        s_out = ctx.enter_context(nc.semaphore("s_out"))
        block = ctx.enter_context(nc.Block())

        def in_sl(g):
            a = (g % IBG) * FREE
            return in_t[:, a : a + FREE]

        def ot_sl(g):
            a = (g % OTG) * FREE
            return ot_t[:, a : a + FREE]

        # out-chunk index containing group g
        def out_chunk_of(g):
            for j, (s, c) in enumerate(zip(OUT_STARTS, OUT_CHUNKS)):
                if s <= g < s + c:
                    return j
            raise AssertionError(g)

        def wait_add(eng, g_prev):
            eng.wait_ge(s_add_e if g_prev % 2 == 0 else s_add_o, g_prev // 2 + 1)

        def issue_in(eng, j):
            gs, c = IN_STARTS[j], IN_CHUNKS[j]
            (sz, idx) = IN_REFS[j]
            a = (gs % IBG) * FREE
            eng.dma_start(
                out=in_t[:, a : a + c * FREE], in_=x_cls[sz][idx]
            ).then_inc(s_in[j % LIN], 16)

        def issue_out(eng, j):
            gs, c = OUT_STARTS[j], OUT_CHUNKS[j]
            ge = gs + c
            eng.wait_ge(s_add_e, (ge + 1) // 2)
            eng.wait_ge(s_add_o, ge // 2)
            (sz, idx) = OUT_REFS[j]
            a = (gs % OTG) * FREE
            eng.dma_start(
                out=o_cls[sz][idx], in_=ot_t[:, a : a + c * FREE]
            ).then_inc(s_out, 16)

        @block.sync
        def _(sync):
            sync.dma_start(out=w_sb[:], in_=w_d[:]).then_inc(s_const, 16)
            sync.dma_start(out=bias_sb[:], in_=b_d[:]).then_inc(s_const, 16)
            for j in range(len(IN_CHUNKS)):
                if j not in GP_IN:
                    issue_in(sync, j)
            for j in range(len(OUT_CHUNKS)):
                if j not in GP_OUT:
                    issue_out(sync, j)

        @block.tensor
        def _(pe):
            pe.wait_ge(s_const, 32)
            for g in range(G):
                if g in IN_STARTS:
                    j = IN_STARTS.index(g)
                    pe.wait_ge(s_in[j % LIN], 16 * (j // LIN + 1))
                if g >= OB:
                    wait_add(pe, g - OB)
                s = g // GPS
                nc.tensor.matmul(
                    op[g % OB][:],
                    lhsT=w_sb[:, s * 128 : (s + 1) * 128],
                    rhs=in_sl(g),
                    start=True,
                    stop=True,
                ).then_inc(s_mm, 1)

        def add_body(eng, g, emit):
            eng.wait_ge(s_mm, g + 1)
            emit(g // GPS)

        @block.vector
        def _(dve):
            dve.wait_ge(s_const, 32)
            for g in range(0, G, 2):
                add_body(
                    dve,
                    g,
                    lambda s, g=g: nc.vector.tensor_scalar_add(
                        ot_sl(g), op[g % OB][:], bias_sb[:, s : s + 1]
                    ).then_inc(s_add_e, 1),
                )

        @block.scalar
        def _(act):
            import concourse.mybir as mybir

            act.wait_ge(s_const, 32)
            for g in range(1, G, 2):
                add_body(
                    act,
                    g,
                    lambda s, g=g: nc.scalar.activation(
                        ot_sl(g),
                        op[g % OB][:],
                        func=mybir.ActivationFunctionType.Identity,
                        bias=bias_sb[:, s : s + 1],
                        scale=1.0,
                    ).then_inc(s_add_o, 1),
                )

        @block.gpsimd
        def _(gp):
            for j in GP_IN:
                issue_in(gp, j)
            for j in GP_OUT:
                issue_out(gp, j)

    return nc


def _to_bf16(a):
    """f32 contiguous -> bf16 (round-to-nearest-even), fast numpy path."""
    import ml_dtypes

    u = np.ascontiguousarray(a, np.float32).view(np.uint32)
    v = ((u + np.uint32(0x7FFF) + ((u >> np.uint32(16)) & np.uint32(1))) >> np.uint32(16)).astype(
        np.uint16
    )
    return v.view(ml_dtypes.bfloat16)


def _host_factors(x):
    """Per-sample affine factors: kron(I8, vh*std) [128,128] bf16, bias col [128] f32.

    The SVD must run through jax-CPU (jaxlib's LAPACK sgesdd) because the
    reference's output depends on the singular-vector sign conventions of that
    exact implementation.
    """
    import jax
    import jax.numpy as jnp

    cpu = jax.devices("cpu")[0]
    _, svs, vhs = jax.jit(
        lambda a: jnp.linalg.svd(a, full_matrices=False), device=cpu
    )(jax.device_put(x, cpu))
    svs = np.asarray(svs)
    vhs = np.asarray(vhs)

    import ml_dtypes

    ws = np.empty((B, 128, 128), ml_dtypes.bfloat16)
    bs = np.empty((B, 128), np.float32)
    eye8 = np.eye(8, dtype=np.float64)
    for s in range(B):
        Xs = x[s]
        sv, vh = svs[s], vhs[s]
        vh64 = vh.astype(np.float64)
        Mm = vh64 @ vh64
        xbar = Xs.mean(axis=0, dtype=np.float64)
        mean = xbar @ vh64
        e2 = (sv.astype(np.float64) ** 2) @ (Mm**2) / N
        var = np.maximum(e2 - mean**2, 0.0)
        std = np.sqrt(var)
        W = vh64 * std[None, :]
        ws[s] = np.kron(eye8, W).astype(ml_dtypes.bfloat16)
        bs[s] = np.tile(mean, 8).astype(np.float32)
    return ws, bs


def _pack_core(xtb_core):
    """[SPC, 16, N] bf16 -> {size: [n, 128, size*FREE] uint16} per IN_CHUNKS."""
    v = xtb_core.view(np.uint16)
    arrs = {c: np.empty((n, 128, c * FREE), np.uint16) for c, n in IN_COUNTS.items()}
    for j, (gs, c) in enumerate(zip(IN_STARTS, IN_CHUNKS)):
        s, n0 = gs // GPS, (gs % GPS) * (M * FREE)
        seg = v[s][:, n0 : n0 + c * M * FREE]          # [16, c*4096]
        t = seg.reshape(NC, c, M, FREE).transpose(2, 0, 1, 3)  # (m,k,i,f)
        sz, idx = IN_REFS[j]
        arrs[sz][idx] = t.reshape(128, c * FREE)
    return arrs


def _unpack_core(res_core):
    """device outputs -> [SPC, 16, N] uint16 (YT layout)."""
    yt = np.empty((SPC, NC, N), np.uint16)
    for j, (gs, c) in enumerate(zip(OUT_STARTS, OUT_CHUNKS)):
        sz, idx = OUT_REFS[j]
        tile = np.asarray(res_core[f"o{sz}"][idx]).view(np.uint16)  # [128, c*FREE]
        s, n0 = gs // GPS, (gs % GPS) * (M * FREE)
        seg = tile.reshape(M, NC, c, FREE).transpose(1, 2, 0, 3)    # (j,i,m,f)
        yt[s][:, n0 : n0 + c * M * FREE] = seg.reshape(NC, c * M * FREE)
    return yt


def kernel(x):
    global _compiled, LAST_EXEC_NS
    from concourse.bass_utils import run_bass_kernel_spmd

    x = np.ascontiguousarray(np.asarray(x), dtype=np.float32).reshape(B, N, NC)
    ws, bs = _host_factors(x)

    xt = np.ascontiguousarray(x.transpose(0, 2, 1))  # [B, 16, N] f32
    xtb = _to_bf16(xt).reshape(B, NC, N)             # [B, 16, N] bf16

    if _compiled is None:
        _compiled = _build_graph()
    nc = _compiled

    import ml_dtypes

    in_maps = []
    for c in range(CORES):
        s0 = c * SPC
        m = {
            # [128, SPC*128]: sample s's kron at columns s*128..(s+1)*128
            "w": np.ascontiguousarray(
                ws[s0 : s0 + SPC].transpose(1, 0, 2).reshape(128, SPC * 128)
            ),
            "bias": np.ascontiguousarray(bs[s0 : s0 + SPC].T),
        }
        for sz, arr in _pack_core(xtb[s0 : s0 + SPC]).items():
            m[f"x{sz}"] = arr.view(ml_dtypes.bfloat16)
        in_maps.append(m)

    res = run_bass_kernel_spmd(nc, in_maps, core_ids=list(range(CORES)), trace=TRACE)
    LAST_EXEC_NS = res.exec_time_ns

    yt_u = np.empty((B, NC, N), np.uint16)
    for c in range(CORES):
        yt_u[c * SPC : (c + 1) * SPC] = _unpack_core(res.results[c])
    yf = (yt_u.astype(np.uint32) << np.uint32(16)).view(np.float32)  # [B,16,N] f32
    out = np.ascontiguousarray(yf.transpose(0, 2, 1))                # [B,N,16]
    return out.reshape(B, 64, 64, 256)


# revision 28
# speedup vs baseline: 1.0465x; 1.0465x over previous
"""BPCA Unpooling kernel for Trainium2 (8 NeuronCores, data-parallel over batch).

Math per sample s (reference semantics):
    _, s_, vh = svd(X)            # X: [N=65536, 16]
    orig = X @ vh
    out  = orig * std(orig, axis=0) + mean(orig, axis=0)   -> reshape [64,64,256]

Identities (same as the f32 baseline): out = X @ W + mean with W = vh * std,
mean/std computed in closed form from the SVD factors on host.  The SVD runs
on host via jax-CPU (LAPACK sgesdd sign conventions must match the reference).

Device formulation ("Y^T layout"): host pre-transposes X to XT [16, N],
converts to bf16, and packs it into per-core DRAM tiles so that each
[128, 512] sub-tile ("group", 4096 rows) R[(m,k), f] = XT[k, n0 + 512m + f].
A single matmul per group with stationary lhsT = kron(I8, W) gives
    P[(m,j), f] = sum_k W[k,j] X[n0+512m+f, k] = Y[n0+512m+f, j]
so output tiles DMA back to DRAM contiguously in the same packed layout,
which host unpacks to Y [N, 16] f32.

This removes the PE transpose pass and the PSUM->SBUF copy of the f32
baseline, and bf16 in/out halves HBM traffic (the binding constraint):
per core 8 MiB in + 8 MiB out ~= 43 us at the ~390 GB/s/core effective DMA
rate, plus ~9 us fixed NEFF startup.

DMA plan (measured):
  - each DIRECT2D dma_start costs ~0.9 us on the issuing sequencer and each
    DMA completion adds ~0.3 us to ring 15 (last ring of the stripe), so the
    steady-state stream uses big 1 MiB (8-group) DMAs;
  - head and tail use small 2-group (256 KiB) DMAs so the first matmul isn't
    gated on a fat chunk and the drain after the last add is fine-grained;
  - only plain 2D [128, F] tiles (contiguous in DRAM) stripe evenly across
    all 16 rings (3D APs were measured to use half the rings), hence one
    DRAM parameter per chunk-size class;
  - w/bias const DMAs are issued by sync BEFORE the input stream so PE's
    weights arrive with the first input tile;
  - SBUF holds a dedicated [128, 512] slot per group for both streams
    (128 KiB/partition total), so there are no slot-reuse waits and the
    semaphore count stays small.

The bias add + f32->bf16 downcast (PSUM -> SBUF) alternates between the DVE
(tensor_scalar add) and the scalar/ACT engine (activation Identity with a
per-partition bias AP).  The gpsimd engine issues output DMAs.

Raw Bass (explicit per-engine programs + semaphores), as walrus only allows
one attached sync-wait per Matmult.
"""

import sys

import numpy as np

sys.path.insert(0, "/opt/trn_rl_repo")

B = 32
N = 65536
NC = 16
CORES = 8
SPC = B // CORES          # samples per core
GPS = 16                  # groups per sample
G = SPC * GPS             # 64 groups per core
FREE = 512
M = 8                     # 512-row blocks per group

# chunk schedules, in groups (each group = 128 KiB bf16)
IN_CHUNKS = [4, 4, 8, 8, 8, 8, 8, 8, 4, 2, 1, 1]
OUT_CHUNKS = [8, 8, 8, 8, 8, 8, 8, 2, 2, 2, 2]
# Queue assignment: sync issues the input stream, gpsimd the output stream
# (balanced 8 MiB each).  The final out-chunk goes on sync (idle by then)
# so the last two DIRECT2D issues overlap instead of serializing on gpsimd.
GP_IN = []
GP_OUT = list(range(len(OUT_CHUNKS) - 1))
assert sum(IN_CHUNKS) == G and sum(OUT_CHUNKS) == G

IBG = 64   # in-tile group slots: one per group, so no slot-reuse waits
OTG = 64   # out-tile group slots: one per group
OB = 6     # matmul PSUM banks
LIN = 8    # in-DMA completion semaphores (chunks >= 8 apart never overlap)


def _starts(chunks):
    s, out = 0, []
    for c in chunks:
        out.append(s)
        s += c
    return out


IN_STARTS = _starts(IN_CHUNKS)
OUT_STARTS = _starts(OUT_CHUNKS)
for _s, _c in zip(IN_STARTS, IN_CHUNKS):
    assert _s % IBG + _c <= IBG and _s % GPS + _c <= GPS
for _s, _c in zip(OUT_STARTS, OUT_CHUNKS):
    assert _s % OTG + _c <= OTG and _s % GPS + _c <= GPS


def _classes(chunks):
    """chunk list -> {size: count}, and per-chunk (size, index-within-size)."""
    counts, refs = {}, []
    for c in chunks:
        i = counts.get(c, 0)
        refs.append((c, i))
        counts[c] = i + 1
    return counts, refs


IN_COUNTS, IN_REFS = _classes(IN_CHUNKS)
OUT_COUNTS, OUT_REFS = _classes(OUT_CHUNKS)

TRACE = False             # test.py sets this for profiling runs
LAST_EXEC_NS = None       # filled when TRACE

_compiled = None


def _build_graph():
    import concourse.bass as bass
    import concourse.mybir as mybir

    f32 = mybir.dt.float32
    bf16 = mybir.dt.bfloat16

    nc = bass.Bass()

    w_d = nc.declare_dram_parameter("w", [128, SPC * 128], bf16, isOutput=False)
    b_d = nc.declare_dram_parameter("bias", [128, SPC], f32, isOutput=False)
    x_cls = {
        c: nc.declare_dram_parameter(f"x{c}", [n, 128, c * FREE], bf16, isOutput=False)
        for c, n in IN_COUNTS.items()
    }
    o_cls = {
        c: nc.declare_dram_parameter(f"o{c}", [n, 128, c * FREE], bf16, isOutput=True)
        for c, n in OUT_COUNTS.items()
    }

    from contextlib import ExitStack

    with ExitStack() as ctx:
        w_sb = ctx.enter_context(nc.sbuf_tensor([128, SPC * 128], bf16))
        bias_sb = ctx.enter_context(nc.sbuf_tensor([128, SPC], f32))
        in_t = ctx.enter_context(nc.sbuf_tensor([128, IBG * FREE], bf16))
        ot_t = ctx.enter_context(nc.sbuf_tensor([128, OTG * FREE], bf16))
        op = [ctx.enter_context(nc.psum_tensor(f"op{i}", [128, FREE], f32)) for i in range(OB)]
        s_const = ctx.enter_context(nc.semaphore())
        s_mm = ctx.enter_context(nc.semaphore())
        s_add_e = ctx.enter_context(nc.semaphore())
        s_add_o = ctx.enter_context(nc.semaphore())
        s_in = [ctx.enter_context(nc.semaphore(f"s_in{i}")) for i in range(LIN)]
        # out-DMA completions have no waiter (every group has its own slot);
        # a single semaphore absorbs the increments
        s_out = ctx.enter_context(nc.semaphore("s_out"))
        block = ctx.enter_context(nc.Block())

        def in_sl(g):
            a = (g % IBG) * FREE
            return in_t[:, a : a + FREE]

        def ot_sl(g):
            a = (g % OTG) * FREE
            return ot_t[:, a : a + FREE]

        # out-chunk index containing group g
        def out_chunk_of(g):
            for j, (s, c) in enumerate(zip(OUT_STARTS, OUT_CHUNKS)):
                if s <= g < s + c:
                    return j
            raise AssertionError(g)

        def wait_add(eng, g_prev):
            eng.wait_ge(s_add_e if g_prev % 2 == 0 else s_add_o, g_prev // 2 + 1)

        def issue_in(eng, j):
            gs, c = IN_STARTS[j], IN_CHUNKS[j]
            (sz, idx) = IN_REFS[j]
            a = (gs % IBG) * FREE
            eng.dma_start(
                out=in_t[:, a : a + c * FREE], in_=x_cls[sz][idx]
            ).then_inc(s_in[j % LIN], 16)

        def issue_out(eng, j):
            gs, c = OUT_STARTS[j], OUT_CHUNKS[j]
            ge = gs + c
            eng.wait_ge(s_add_e, (ge + 1) // 2)
            eng.wait_ge(s_add_o, ge // 2)
            (sz, idx) = OUT_REFS[j]
            a = (gs % OTG) * FREE
            eng.dma_start(
                out=o_cls[sz][idx], in_=ot_t[:, a : a + c * FREE]
            ).then_inc(s_out, 16)

        @block.sync
        def _(sync):
            # first two data chunks go ahead of the consts: they gate the
            # whole stream, while PE only needs w by its first matmul (~3 us
            # after the first chunk lands)
            issue_in(sync, 0)
            issue_in(sync, 1)
            sync.dma_start(out=w_sb[:], in_=w_d[:]).then_inc(s_const, 16)
            sync.dma_start(out=bias_sb[:], in_=b_d[:]).then_inc(s_const, 16)
            for j in range(2, len(IN_CHUNKS)):
                if j not in GP_IN:
                    issue_in(sync, j)
            for j in range(len(OUT_CHUNKS)):
                if j not in GP_OUT:
                    issue_out(sync, j)

        @block.tensor
        def _(pe):
            pe.wait_ge(s_const, 32)
            for g in range(G):
                if g in IN_STARTS:
                    j = IN_STARTS.index(g)
                    pe.wait_ge(s_in[j % LIN], 16 * (j // LIN + 1))
                if g >= OB:
                    wait_add(pe, g - OB)
                s = g // GPS
                nc.tensor.matmul(
                    op[g % OB][:],
                    lhsT=w_sb[:, s * 128 : (s + 1) * 128],
                    rhs=in_sl(g),
                    start=True,
                    stop=True,
                ).then_inc(s_mm, 1)

        def add_body(eng, g, emit):
            eng.wait_ge(s_mm, g + 1)
            emit(g // GPS)

        @block.vector
        def _(dve):
            dve.wait_ge(s_const, 32)
            for g in range(0, G, 2):
                add_body(
                    dve,
                    g,
                    lambda s, g=g: nc.vector.tensor_scalar_add(
                        ot_sl(g), op[g % OB][:], bias_sb[:, s : s + 1]
                    ).then_inc(s_add_e, 1),
                )

        @block.scalar
        def _(act):
            import concourse.mybir as mybir

            act.wait_ge(s_const, 32)
            for g in range(1, G, 2):
                add_body(
                    act,
                    g,
                    lambda s, g=g: nc.scalar.activation(
                        ot_sl(g),
                        op[g % OB][:],
                        func=mybir.ActivationFunctionType.Identity,
                        bias=bias_sb[:, s : s + 1],
                        scale=1.0,
                    ).then_inc(s_add_o, 1),
                )

        @block.gpsimd
        def _(gp):
            for j in GP_IN:
                issue_in(gp, j)
            for j in GP_OUT:
                issue_out(gp, j)

    return nc


def _to_bf16(a):
    """f32 contiguous -> bf16 (round-to-nearest-even), fast numpy path."""
    import ml_dtypes

    u = np.ascontiguousarray(a, np.float32).view(np.uint32)
    v = ((u + np.uint32(0x7FFF) + ((u >> np.uint32(16)) & np.uint32(1))) >> np.uint32(16)).astype(
        np.uint16
    )
    return v.view(ml_dtypes.bfloat16)


def _host_factors(x):
    """Per-sample affine factors: kron(I8, vh*std) [128,128] bf16, bias col [128] f32.

    The SVD must run through jax-CPU (jaxlib's LAPACK sgesdd) because the
    reference's output depends on the singular-vector sign conventions of that
    exact implementation.
    """
    import jax
    import jax.numpy as jnp

    cpu = jax.devices("cpu")[0]
    _, svs, vhs = jax.jit(
        lambda a: jnp.linalg.svd(a, full_matrices=False), device=cpu
    )(jax.device_put(x, cpu))
    svs = np.asarray(svs)
    vhs = np.asarray(vhs)

    import ml_dtypes

    ws = np.empty((B, 128, 128), ml_dtypes.bfloat16)
    bs = np.empty((B, 128), np.float32)
    eye8 = np.eye(8, dtype=np.float64)
    for s in range(B):
        Xs = x[s]
        sv, vh = svs[s], vhs[s]
        vh64 = vh.astype(np.float64)
        Mm = vh64 @ vh64
        xbar = Xs.mean(axis=0, dtype=np.float64)
        mean = xbar @ vh64
        e2 = (sv.astype(np.float64) ** 2) @ (Mm**2) / N
        var = np.maximum(e2 - mean**2, 0.0)
        std = np.sqrt(var)
        W = vh64 * std[None, :]
        ws[s] = np.kron(eye8, W).astype(ml_dtypes.bfloat16)
        bs[s] = np.tile(mean, 8).astype(np.float32)
    return ws, bs


def _pack_core(xtb_core):
    """[SPC, 16, N] bf16 -> {size: [n, 128, size*FREE] uint16} per IN_CHUNKS."""
    v = xtb_core.view(np.uint16)
    arrs = {c: np.empty((n, 128, c * FREE), np.uint16) for c, n in IN_COUNTS.items()}
    for j, (gs, c) in enumerate(zip(IN_STARTS, IN_CHUNKS)):
        s, n0 = gs // GPS, (gs % GPS) * (M * FREE)
        seg = v[s][:, n0 : n0 + c * M * FREE]          # [16, c*4096]
        t = seg.reshape(NC, c, M, FREE).transpose(2, 0, 1, 3)  # (m,k,i,f)
        sz, idx = IN_REFS[j]
        arrs[sz][idx] = t.reshape(128, c * FREE)
    return arrs


def _unpack_core(res_core):
    """device outputs -> [SPC, 16, N] uint16 (YT layout)."""
    yt = np.empty((SPC, NC, N), np.uint16)
    for j, (gs, c) in enumerate(zip(OUT_STARTS, OUT_CHUNKS)):
        sz, idx = OUT_REFS[j]
        tile = np.asarray(res_core[f"o{sz}"][idx]).view(np.uint16)  # [128, c*FREE]
        s, n0 = gs // GPS, (gs % GPS) * (M * FREE)
        seg = tile.reshape(M, NC, c, FREE).transpose(1, 2, 0, 3)    # (j,i,m,f)
        yt[s][:, n0 : n0 + c * M * FREE] = seg.reshape(NC, c * M * FREE)
    return yt


def kernel(x):
    global _compiled, LAST_EXEC_NS
    from concourse.bass_utils import run_bass_kernel_spmd

    x = np.ascontiguousarray(np.asarray(x), dtype=np.float32).reshape(B, N, NC)
    ws, bs = _host_factors(x)

    xt = np.ascontiguousarray(x.transpose(0, 2, 1))  # [B, 16, N] f32
    xtb = _to_bf16(xt).reshape(B, NC, N)             # [B, 16, N] bf16

    if _compiled is None:
        _compiled = _build_graph()
    nc = _compiled

    import ml_dtypes

    in_maps = []
    for c in range(CORES):
        s0 = c * SPC
        m = {
            # [128, SPC*128]: sample s's kron at columns s*128..(s+1)*128
            "w": np.ascontiguousarray(
                ws[s0 : s0 + SPC].transpose(1, 0, 2).reshape(128, SPC * 128)
            ),
            "bias": np.ascontiguousarray(bs[s0 : s0 + SPC].T),
        }
        for sz, arr in _pack_core(xtb[s0 : s0 + SPC]).items():
            m[f"x{sz}"] = arr.view(ml_dtypes.bfloat16)
        in_maps.append(m)

    res = run_bass_kernel_spmd(nc, in_maps, core_ids=list(range(CORES)), trace=TRACE)
    LAST_EXEC_NS = res.exec_time_ns

    yt_u = np.empty((B, NC, N), np.uint16)
    for c in range(CORES):
        yt_u[c * SPC : (c + 1) * SPC] = _unpack_core(res.results[c])
    yf = (yt_u.astype(np.uint32) << np.uint32(16)).view(np.float32)  # [B,16,N] f32
    out = np.ascontiguousarray(yf.transpose(0, 2, 1))                # [B,N,16]
    return out.reshape(B, 64, 64, 256)
